# revision 1
# baseline (speedup 1.0000x reference)
"""EpisodicMemory Trainium2 kernel (8 NeuronCores, pure data parallel over batch).

Reference semantics (per batch b):
    keys_w   = keys   with row write_ptr[b] <- key[b]
    values_w = values with row write_ptr[b] <- value[b]
    filled_w = min(filled + 1, S)
    query    = hidden @ Wq.T + bq
    scores   = (keys_w @ query) / sqrt(K), masked to s < filled_w
    attn     = softmax(scores)
    retrieved= attn @ values_w
    g        = silu([hidden|retrieved] @ Wg1.T + bg1)
    gate     = sigmoid(g @ Wg2.T + bg2)
    out      = (hidden + gate*retrieved) @ Wo.T + bo

The scatter is never materialized: base scores/retrieved are computed from the
original keys/values and corrected algebraically with the gathered old rows at
write_ptr (indirect DMA) plus the new key/value rows.
"""

import sys

sys.path.insert(0, "/opt/trn_rl_repo")

import numpy as np

import concourse.bacc as bacc
import concourse.tile as tile
from concourse import bass, mybir
from concourse.bass_utils import run_bass_kernel_spmd
from concourse.masks import make_identity

B, S, K, V = 512, 1024, 128, 512
NCORES = 8
NB = B // NCORES          # 64 batches per core
T = S // 128              # 8 s-chunks of 128
GRP = 16                  # batches per softmax group
NG = NB // GRP            # 4 groups
SCALE = float(np.sqrt(K))
NEG_BIG = -3.0e37

F32 = mybir.dt.float32
I32 = mybir.dt.int32

# dtype used for the attn @ values matvec (the PE-heavy part)
VALUES_MM_DTYPE = mybir.dt.float32r

# debug stubs (empty for production): 'noind','noqrows','nostitch','nogrow','novals','noscores'
_STUBS = set()


def _build():
    nc = bacc.Bacc()
    dt = F32

    # ---- DRAM tensors (per-core shard) ----
    keys_t = nc.dram_tensor("keys", [NB, S, K], dt, kind="ExternalInput")
    values_t = nc.dram_tensor("values", [NB, S, V], VALUES_MM_DTYPE, kind="ExternalInput")
    key_t = nc.dram_tensor("key", [NB, K], dt, kind="ExternalInput")
    value_t = nc.dram_tensor("value", [NB, V], dt, kind="ExternalInput")
    hidden_t = nc.dram_tensor("hidden", [NB, V], dt, kind="ExternalInput")
    filled_t = nc.dram_tensor("filled_f", [NB, 1], dt, kind="ExternalInput")
    wp_t = nc.dram_tensor("wp_f", [NB, 1], dt, kind="ExternalInput")
    rowidx_t = nc.dram_tensor("row_idx", [NB, 1], I32, kind="ExternalInput")
    wqT_t = nc.dram_tensor("WqT", [V, K], dt, kind="ExternalInput")       # Wq.T
    wg1T_t = nc.dram_tensor("Wg1T", [2 * V, V], dt, kind="ExternalInput")  # Wg1.T
    wg2T_t = nc.dram_tensor("Wg2T", [V, V], dt, kind="ExternalInput")     # Wg2.T
    woT_t = nc.dram_tensor("WoT", [V, V], dt, kind="ExternalInput")       # Wo.T
    bq_t = nc.dram_tensor("bq", [K], dt, kind="ExternalInput")
    bg1_t = nc.dram_tensor("bg1", [V], dt, kind="ExternalInput")
    bg2_t = nc.dram_tensor("bg2", [V], dt, kind="ExternalInput")
    bo_t = nc.dram_tensor("bo", [V], dt, kind="ExternalInput")
    out_t = nc.dram_tensor("out", [NB, V], dt, kind="ExternalOutput")

    keys_view = keys_t[:].rearrange("b (p t) k -> b p t k", p=128)
    values_view = values_t[:].rearrange("b (p t) v -> b p t v", p=128)
    keys_rows = keys_t[:].rearrange("b s k -> (b s) k")
    values_rows = values_t[:].rearrange("b s v -> (b s) v")

    with tile.TileContext(nc) as tc:
        with (
            tc.tile_pool(name="const", bufs=1) as const,
            tc.tile_pool(name="ktile", bufs=3) as ktile_p,
            tc.tile_pool(name="vtile", bufs=5) as vtile_p,
            tc.tile_pool(name="grp", bufs=2) as grp_p,
            tc.tile_pool(name="qr", bufs=1) as qr_p,
            tc.tile_pool(name="sm", bufs=1) as sm_p,
            tc.tile_pool(name="grow", bufs=3) as grow_p,
            tc.tile_pool(name="misc", bufs=1) as misc,
            tc.tile_pool(name="ps_qb", bufs=2, space="PSUM") as ps_qb,
            tc.tile_pool(name="ps_tr", bufs=2, space="PSUM") as ps_tr,
            tc.tile_pool(name="ps_g", bufs=4, space="PSUM") as ps_g,
        ):
            # ---------------- setup ----------------
            identity = const.tile([128, 128], dt)
            make_identity(nc, identity[:])
            ones_row = const.tile([1, 128], dt)
            nc.vector.memset(ones_row[:], 1.0)

            iota_i = ktile_p.tile([GRP, S], mybir.dt.int16, tag="ktile")
            nc.gpsimd.iota(iota_i[:], pattern=[[1, S]], base=0, channel_multiplier=0)
            iota_f = const.tile([GRP, S], dt)
            nc.vector.tensor_copy(out=iota_f[:], in_=iota_i[:])

            wqT = const.tile([128, 4, K], dt)
            nc.scalar.dma_start(out=wqT[:], in_=wqT_t[:].rearrange("(c p) k -> p c k", p=128))
            wg1T = const.tile([128, 8, V], dt)
            nc.scalar.dma_start(out=wg1T[:], in_=wg1T_t[:].rearrange("(c p) j -> p c j", p=128))
            wg2T = const.tile([128, 4, V], dt)
            nc.scalar.dma_start(out=wg2T[:], in_=wg2T_t[:].rearrange("(c p) j -> p c j", p=128))
            woT = const.tile([128, 4, V], dt)
            nc.scalar.dma_start(out=woT[:], in_=woT_t[:].rearrange("(c p) j -> p c j", p=128))
            bq_row = const.tile([1, K], dt)
            nc.scalar.dma_start(out=bq_row[:], in_=bq_t[None, :])
            bg1_row = const.tile([1, V], dt)
            nc.scalar.dma_start(out=bg1_row[:], in_=bg1_t[None, :])
            bg2_row = const.tile([1, V], dt)
            nc.scalar.dma_start(out=bg2_row[:], in_=bg2_t[None, :])
            bo_row = const.tile([1, V], dt)
            nc.scalar.dma_start(out=bo_row[:], in_=bo_t[None, :])

            hidden_sb = misc.tile([NB, V], dt)
            nc.scalar.dma_start(out=hidden_sb[:], in_=hidden_t[:, :])
            key_sb = misc.tile([NB, K], dt)
            nc.scalar.dma_start(out=key_sb[:], in_=key_t[:, :])
            value_sb = misc.tile([NB, V], dt)
            nc.scalar.dma_start(out=value_sb[:], in_=value_t[:, :])
            filled_sb = misc.tile([NB, 1], dt)
            nc.scalar.dma_start(out=filled_sb[:], in_=filled_t[:, :])
            wp_sb = misc.tile([NB, 1], dt)
            nc.scalar.dma_start(out=wp_sb[:], in_=wp_t[:, :])
            rowidx_sb = misc.tile([NB, 1], I32)
            nc.scalar.dma_start(out=rowidx_sb[:], in_=rowidx_t[:, :])

            # gather the pre-scatter rows at write_ptr
            kwp_sb = misc.tile([NB, K], dt)
            vwp_sb = misc.tile([NB, V], dt)
            if "noind" in _STUBS:
                nc.vector.memset(kwp_sb[:], 0.0)
                nc.vector.memset(vwp_sb[:], 0.0)
            else:
                nc.gpsimd.indirect_dma_start(
                    out=kwp_sb[:], out_offset=None, in_=keys_rows,
                    in_offset=bass.IndirectOffsetOnAxis(ap=rowidx_sb[:, :1], axis=0),
                )
                nc.gpsimd.indirect_dma_start(
                    out=vwp_sb[:], out_offset=None, in_=values_rows,
                    in_offset=bass.IndirectOffsetOnAxis(ap=rowidx_sb[:, :1], axis=0),
                )

            # hiddenT (128v x 64b) chunks
            hT = misc.tile([128, 4, NB], dt)
            for c in range(4):
                tp = ps_tr.tile([128, NB], dt, tag="tr")
                nc.tensor.transpose(out=tp[:], in_=hidden_sb[:, c * 128:(c + 1) * 128], identity=identity[:NB, :NB])
                nc.scalar.copy(out=hT[:, c, :], in_=tp[:])

            # query = hidden @ Wq.T + bq  -> (64b x 128k)
            q_ps = ps_tr.tile([NB, K], dt, tag="tr")
            for c in range(4):
                nc.tensor.matmul(out=q_ps[:], lhsT=hT[:, c, :], rhs=wqT[:, c, :],
                                 start=(c == 0), stop=False)
            nc.tensor.matmul(out=q_ps[:], lhsT=ones_row[:, :NB], rhs=bq_row[:],
                             start=False, stop=True)
            query_sb = misc.tile([NB, K], dt)
            nc.vector.tensor_copy(out=query_sb[:], in_=q_ps[:])

            # raw (unscaled) dot(key_row, query) for old/new rows at write_ptr
            junk_rd = misc.tile([NB, K], dt)
            sold = misc.tile([NB, 1], dt)
            nc.vector.tensor_mul(out=junk_rd[:], in0=kwp_sb[:], in1=query_sb[:])
            nc.vector.tensor_reduce(out=sold[:], in_=junk_rd[:],
                                    axis=mybir.AxisListType.X, op=mybir.AluOpType.add)
            snew = misc.tile([NB, 1], dt)
            nc.vector.tensor_mul(out=junk_rd[:], in0=key_sb[:], in1=query_sb[:])
            nc.vector.tensor_reduce(out=snew[:], in_=junk_rd[:],
                                    axis=mybir.AxisListType.X, op=mybir.AluOpType.add)

            denom0 = misc.tile([NB, 1], dt)
            neg_m_all = misc.tile([NB, 1], dt)
            attnT_groups = []
            g_sb = misc.tile([NB, V], dt)

            prod_s = misc.tile([128, T, K], dt)

            def scores_stage(g):
                b0 = g * GRP
                # query rows of this group -> partition 0 free-dim layout
                qrows = qr_p.tile([1, GRP * K], dt, tag="qrows")
                if "noqrows" in _STUBS:
                    nc.vector.memset(qrows[:], 0.01)
                else:
                    nc.gpsimd.dma_start(
                        out=qrows[:].rearrange("p (b k) -> p b k", b=GRP),
                        in_=query_sb[b0:b0 + GRP, None, :])
                filled_g = qr_p.tile([GRP, 1], dt, tag="filled_g")
                nc.gpsimd.dma_start(out=filled_g[:], in_=filled_t[b0:b0 + GRP, :])
                penalty_g = sm_p.tile([GRP, S], dt, tag="penalty_g")
                nc.vector.tensor_scalar(
                    out=penalty_g[:], in0=iota_f[:], scalar1=filled_g[:, :1],
                    scalar2=NEG_BIG, op0=mybir.AluOpType.is_ge, op1=mybir.AluOpType.mult)

                sT = grp_p.tile([128, T, GRP], dt, tag="sT")
                for bl in range(GRP):
                    b = b0 + bl
                    kt = ktile_p.tile([128, T, K], dt, tag="ktile")
                    nc.gpsimd.dma_start(out=kt[:], in_=keys_view[b])
                    qb = ps_qb.tile([128, 128], dt, tag="qb")
                    nc.tensor.matmul(out=qb[:], lhsT=ones_row[:],
                                     rhs=qrows[:, bl * K:(bl + 1) * K],
                                     start=True, stop=True)
                    qb_sb = ktile_p.tile([128, 128], dt, tag="qb_sb")
                    nc.scalar.copy(out=qb_sb[:], in_=qb[:])
                    qb_ap = qb_sb[:]
                    qb_bcast = bass.AP(tensor=qb_ap.tensor, offset=qb_ap.offset,
                                       ap=[qb_ap.ap[0], [0, T], qb_ap.ap[1]])
                    nc.vector.tensor_tensor(out=prod_s[:], in0=kt[:], in1=qb_bcast,
                                            op=mybir.AluOpType.mult)
                    nc.vector.tensor_reduce(out=sT[:, :, bl], in_=prod_s[:],
                                            axis=mybir.AxisListType.X,
                                            op=mybir.AluOpType.add)

                # transpose score columns back to rows, add the -inf penalty
                scores_g = sm_p.tile([GRP, S], dt, tag="scores_g")
                scores_v = scores_g[:].rearrange("g (x t) -> g x t", t=T)
                penalty_v = penalty_g[:].rearrange("g (x t) -> g x t", t=T)
                for t in range(T):
                    tp = ps_tr.tile([GRP, 128], dt, tag="tr")
                    nc.tensor.transpose(out=tp[:], in_=sT[:, t, :], identity=identity[:])
                    nc.vector.tensor_tensor(
                        out=scores_v[:, :, t], in0=tp[:],
                        in1=penalty_v[:, :, t],
                        op=mybir.AluOpType.add)

                m_g = sm_p.tile([GRP, 1], dt, tag="m_g")
                nc.vector.tensor_reduce(out=m_g[:], in_=scores_g[:],
                                        axis=mybir.AxisListType.X,
                                        op=mybir.AluOpType.max)
                neg_m_g = sm_p.tile([GRP, 1], dt, tag="neg_m_g")
                nc.scalar.mul(out=neg_m_g[:], in_=m_g[:], mul=-1.0 / SCALE)
                exps_g = sm_p.tile([GRP, S], dt, tag="exps_g")
                denom0_g = sm_p.tile([GRP, 1], dt, tag="denom0_g")
                nc.scalar.activation(
                    out=exps_g[:], in_=scores_g[:],
                    func=mybir.ActivationFunctionType.Exp,
                    bias=neg_m_g[:, :1], scale=1.0 / SCALE,
                    accum_out=denom0_g[:, :1])

                attnT = grp_p.tile([128, T, GRP], VALUES_MM_DTYPE, tag="attnT")
                exps_v = exps_g[:].rearrange("g (x t) -> g x t", t=T)
                for t in range(T):
                    tp = ps_tr.tile([128, GRP], dt, tag="tr")
                    nc.tensor.transpose(out=tp[:],
                                        in_=exps_v[:, :, t],
                                        identity=identity[:GRP, :GRP])
                    nc.scalar.copy(out=attnT[:, t, :], in_=tp[:])
                attnT_groups.append(attnT)

                # stitch per-group scalars into the global (NB,1) tiles
                if "nostitch" not in _STUBS:
                    nc.gpsimd.dma_start(out=denom0[b0:b0 + GRP, :], in_=denom0_g[:])
                    nc.gpsimd.dma_start(out=neg_m_all[b0:b0 + GRP, :], in_=neg_m_g[:])

            def values_stage(g):
                b0 = g * GRP
                attnT = attnT_groups[g]
                for bl in range(GRP):
                    b = b0 + bl
                    vt = vtile_p.tile([128, T, V], VALUES_MM_DTYPE, tag="vtile")
                    nc.sync.dma_start(out=vt[:], in_=values_view[b])
                    g_ps = ps_g.tile([1, V], dt, tag="g_ps")
                    for t in range(T):
                        nc.tensor.matmul(out=g_ps[:], lhsT=attnT[:, t, bl:bl + 1],
                                         rhs=vt[:, t, :],
                                         start=(t == 0), stop=(t == T - 1))
                    g_row = grow_p.tile([1, V], dt, tag="g_row")
                    nc.scalar.copy(out=g_row[:], in_=g_ps[:])
                    if "nogrow" not in _STUBS:
                        nc.gpsimd.dma_start(out=g_sb[b:b + 1, :], in_=g_row[:])

            if "nostitch" in _STUBS:
                nc.vector.memset(denom0[:], 1.0)
                nc.vector.memset(neg_m_all[:], 0.0)
            if "nogrow" in _STUBS or "novals" in _STUBS:
                nc.vector.memset(g_sb[:], 0.0)
            for g in range(NG):
                if g > 0 and "novals" not in _STUBS:
                    values_stage(g - 1)
                scores_stage(g)
            if "novals" not in _STUBS:
                values_stage(NG - 1)

            # ---------------- corrections + softmax denominator ----------------
            eo = misc.tile([NB, 1], dt)
            nc.scalar.activation(out=eo[:], in_=sold[:],
                                 func=mybir.ActivationFunctionType.Exp,
                                 bias=neg_m_all[:, :1], scale=1.0 / SCALE)
            en = misc.tile([NB, 1], dt)
            nc.scalar.activation(out=en[:], in_=snew[:],
                                 func=mybir.ActivationFunctionType.Exp,
                                 bias=neg_m_all[:, :1], scale=1.0 / SCALE)
            mask_wp = misc.tile([NB, 1], dt)
            nc.vector.tensor_tensor(out=mask_wp[:], in0=wp_sb[:], in1=filled_sb[:],
                                    op=mybir.AluOpType.is_lt)
            a_old = misc.tile([NB, 1], dt)
            nc.vector.tensor_mul(out=a_old[:], in0=eo[:], in1=mask_wp[:])
            a_new = misc.tile([NB, 1], dt)
            nc.vector.tensor_mul(out=a_new[:], in0=en[:], in1=mask_wp[:])
            denom = misc.tile([NB, 1], dt)
            nc.vector.tensor_sub(out=denom[:], in0=denom0[:], in1=a_old[:])
            nc.vector.tensor_add(out=denom[:], in0=denom[:], in1=a_new[:])
            recip = misc.tile([NB, 1], dt)
            nc.vector.reciprocal(out=recip[:], in_=denom[:])

            # retrieved = (G + a_new*value - a_old*values[wp]) / denom
            t1 = misc.tile([NB, V], dt)
            nc.vector.tensor_scalar_mul(out=t1[:], in0=value_sb[:], scalar1=a_new[:, :1])
            t2 = misc.tile([NB, V], dt)
            nc.vector.tensor_scalar_mul(out=t2[:], in0=vwp_sb[:], scalar1=a_old[:, :1])
            nc.vector.tensor_sub(out=t1[:], in0=t1[:], in1=t2[:])
            nc.vector.tensor_add(out=t1[:], in0=g_sb[:], in1=t1[:])
            retr = misc.tile([NB, V], dt)
            nc.vector.tensor_scalar_mul(out=retr[:], in0=t1[:], scalar1=recip[:, :1])

            # ---------------- MLP ----------------
            rT = misc.tile([128, 4, NB], dt)
            for c in range(4):
                tp = ps_tr.tile([128, NB], dt, tag="tr")
                nc.tensor.transpose(out=tp[:], in_=retr[:, c * 128:(c + 1) * 128],
                                    identity=identity[:NB, :NB])
                nc.scalar.copy(out=rT[:, c, :], in_=tp[:])

            g_ps = ps_tr.tile([NB, V], dt, tag="tr")
            for ic in range(8):
                lhsT = hT[:, ic, :] if ic < 4 else rT[:, ic - 4, :]
                nc.tensor.matmul(out=g_ps[:], lhsT=lhsT, rhs=wg1T[:, ic, :],
                                 start=(ic == 0), stop=False)
            nc.tensor.matmul(out=g_ps[:], lhsT=ones_row[:, :NB], rhs=bg1_row[:],
                             start=False, stop=True)
            g_act = misc.tile([NB, V], dt)
            nc.scalar.activation(out=g_act[:], in_=g_ps[:],
                                 func=mybir.ActivationFunctionType.Sigmoid)
            nc.vector.tensor_mul(out=g_act[:], in0=g_act[:], in1=g_ps[:])

            gT = misc.tile([128, 4, NB], dt)
            for c in range(4):
                tp = ps_tr.tile([128, NB], dt, tag="tr")
                nc.tensor.transpose(out=tp[:], in_=g_act[:, c * 128:(c + 1) * 128],
                                    identity=identity[:NB, :NB])
                nc.scalar.copy(out=gT[:, c, :], in_=tp[:])

            gate_ps = ps_tr.tile([NB, V], dt, tag="tr")
            for c in range(4):
                nc.tensor.matmul(out=gate_ps[:], lhsT=gT[:, c, :], rhs=wg2T[:, c, :],
                                 start=(c == 0), stop=False)
            nc.tensor.matmul(out=gate_ps[:], lhsT=ones_row[:, :NB], rhs=bg2_row[:],
                             start=False, stop=True)
            gate = misc.tile([NB, V], dt)
            nc.scalar.activation(out=gate[:], in_=gate_ps[:],
                                 func=mybir.ActivationFunctionType.Sigmoid)

            z = misc.tile([NB, V], dt)
            nc.vector.tensor_mul(out=z[:], in0=gate[:], in1=retr[:])
            nc.vector.tensor_add(out=z[:], in0=z[:], in1=hidden_sb[:])

            zT = misc.tile([128, 4, NB], dt)
            for c in range(4):
                tp = ps_tr.tile([128, NB], dt, tag="tr")
                nc.tensor.transpose(out=tp[:], in_=z[:, c * 128:(c + 1) * 128],
                                    identity=identity[:NB, :NB])
                nc.scalar.copy(out=zT[:, c, :], in_=tp[:])

            out_ps = ps_tr.tile([NB, V], dt, tag="tr")
            for c in range(4):
                nc.tensor.matmul(out=out_ps[:], lhsT=zT[:, c, :], rhs=woT[:, c, :],
                                 start=(c == 0), stop=False)
            nc.tensor.matmul(out=out_ps[:], lhsT=ones_row[:, :NB], rhs=bo_row[:],
                             start=False, stop=True)
            out_sb = misc.tile([NB, V], dt)
            nc.vector.tensor_copy(out=out_sb[:], in_=out_ps[:])
            nc.sync.dma_start(out=out_t[:, :], in_=out_sb[:])

    nc.finalize()
    return nc


_NC_CACHE = None


def _get_nc():
    global _NC_CACHE
    if _NC_CACHE is None:
        _NC_CACHE = _build()
    return _NC_CACHE


def _make_in_maps(keys, values, key, value, hidden, write_ptr, filled,
                  Wq, bq, Wg1, bg1, Wg2, bg2, Wo, bo):
    f32 = np.float32
    keys = np.ascontiguousarray(np.asarray(keys, dtype=f32))
    values = np.ascontiguousarray(np.asarray(values, dtype=f32))
    key = np.ascontiguousarray(np.asarray(key, dtype=f32))
    value = np.ascontiguousarray(np.asarray(value, dtype=f32))
    hidden = np.ascontiguousarray(np.asarray(hidden, dtype=f32))
    wp = np.asarray(write_ptr).astype(np.int64)
    fl = np.asarray(filled).astype(np.int64)

    wqT = np.ascontiguousarray(np.asarray(Wq, dtype=f32).T)
    wg1T = np.ascontiguousarray(np.asarray(Wg1, dtype=f32).T)
    wg2T = np.ascontiguousarray(np.asarray(Wg2, dtype=f32).T)
    woT = np.ascontiguousarray(np.asarray(Wo, dtype=f32).T)
    bq = np.ascontiguousarray(np.asarray(bq, dtype=f32))
    bg1 = np.ascontiguousarray(np.asarray(bg1, dtype=f32))
    bg2 = np.ascontiguousarray(np.asarray(bg2, dtype=f32))
    bo = np.ascontiguousarray(np.asarray(bo, dtype=f32))

    filled_w = np.minimum(fl + 1, S).astype(f32).reshape(B, 1)
    wp_f = wp.astype(f32).reshape(B, 1)

    in_maps = []
    for c in range(NCORES):
        sl = slice(c * NB, (c + 1) * NB)
        wp_c = wp[sl]
        row_idx = (np.arange(NB, dtype=np.int64) * S + wp_c).astype(np.int32)
        in_maps.append({
            "keys": keys[sl],
            "values": values[sl],
            "key": key[sl],
            "value": value[sl],
            "hidden": hidden[sl],
            "filled_f": filled_w[sl],
            "wp_f": wp_f[sl],
            "row_idx": row_idx.reshape(NB, 1),
            "WqT": wqT, "Wg1T": wg1T, "Wg2T": wg2T, "WoT": woT,
            "bq": bq, "bg1": bg1, "bg2": bg2, "bo": bo,
        })
    return in_maps


def run(trace=False, **inputs):
    nc = _get_nc()
    in_maps = _make_in_maps(**inputs)
    res = run_bass_kernel_spmd(nc, in_maps, core_ids=list(range(NCORES)),
                               trace=trace)
    out = np.concatenate([res.results[c]["out"] for c in range(NCORES)], axis=0)
    return out, res


def kernel(**inputs) -> np.ndarray:
    out, _ = run(trace=False, **inputs)
    return out



# revision 11
# speedup vs baseline: 1.6865x; 1.6865x over previous
"""EpisodicMemory Trainium2 kernel (8 NeuronCores, pure data parallel over batch).

Reference semantics (per batch b):
    keys_w   = keys   with row write_ptr[b] <- key[b]
    values_w = values with row write_ptr[b] <- value[b]
    filled_w = min(filled + 1, S)
    query    = hidden @ Wq.T + bq
    scores   = (keys_w @ query) / sqrt(K), masked to s < filled_w
    attn     = softmax(scores)
    retrieved= attn @ values_w
    g        = silu([hidden|retrieved] @ Wg1.T + bg1)
    gate     = sigmoid(g @ Wg2.T + bg2)
    out      = (hidden + gate*retrieved) @ Wo.T + bo

The scatter is never materialized: base scores/retrieved are computed from the
original keys/values and corrected algebraically with the gathered old rows at
write_ptr (indirect DMA) plus the new key/value rows.
"""

import sys

sys.path.insert(0, "/opt/trn_rl_repo")

import ml_dtypes
import numpy as np

import concourse.bacc as bacc
import concourse.tile as tile
from concourse import bass, mybir
from concourse.bass_utils import run_bass_kernel_spmd
from concourse.masks import make_identity

B, S, K, V = 512, 1024, 128, 512
NCORES = 8
NB = B // NCORES          # 64 batches per core
T = S // 128              # 8 s-chunks of 128
GRP = 16                  # batches per softmax group
NG = NB // GRP            # 4 groups
SCALE = float(np.sqrt(K))
NEG_BIG = -3.0e37

F32 = mybir.dt.float32
I32 = mybir.dt.int32
BF16 = mybir.dt.bfloat16
FP8 = mybir.dt.float8e4

# dtype used for the attn @ values matvec (the PE-heavy part)
VALUES_MM_DTYPE = FP8

# debug stubs (empty for production): 'noind','noqrows','nostitch','nogrow','novals','noscores'
_STUBS = set()


def _build():
    nc = bacc.Bacc()
    dt = F32

    # ---- DRAM tensors (per-core shard) ----
    keys_t = nc.dram_tensor("keys", [NB, S, K], FP8, kind="ExternalInput")
    values_t = nc.dram_tensor("values", [NB, S, V], FP8, kind="ExternalInput")
    key_t = nc.dram_tensor("key", [NB, K], dt, kind="ExternalInput")
    value_t = nc.dram_tensor("value", [NB, V], dt, kind="ExternalInput")
    hidden_t = nc.dram_tensor("hidden", [NB, V], dt, kind="ExternalInput")
    filled_t = nc.dram_tensor("filled_f", [NB, 1], dt, kind="ExternalInput")
    wp_t = nc.dram_tensor("wp_f", [NB, 1], dt, kind="ExternalInput")
    rowidx_t = nc.dram_tensor("row_idx", [NB, 1], I32, kind="ExternalInput")
    wqT_t = nc.dram_tensor("WqT", [V, K], dt, kind="ExternalInput")       # Wq.T
    wg1T_t = nc.dram_tensor("Wg1T", [2 * V, V], dt, kind="ExternalInput")  # Wg1.T
    wg2T_t = nc.dram_tensor("Wg2T", [V, V], dt, kind="ExternalInput")     # Wg2.T
    woT_t = nc.dram_tensor("WoT", [V, V], dt, kind="ExternalInput")       # Wo.T
    bq_t = nc.dram_tensor("bq", [K], dt, kind="ExternalInput")
    bg1_t = nc.dram_tensor("bg1", [V], dt, kind="ExternalInput")
    bg2_t = nc.dram_tensor("bg2", [V], dt, kind="ExternalInput")
    bo_t = nc.dram_tensor("bo", [V], dt, kind="ExternalInput")
    out_t = nc.dram_tensor("out", [NB, V], dt, kind="ExternalOutput")

    keys_view = keys_t[:].rearrange("b (p t) k -> b p t k", p=128)
    values_view = values_t[:].rearrange("b (p t) v -> b p t v", p=128)
    keys_rows = keys_t[:].rearrange("b s k -> (b s) k")
    values_rows = values_t[:].rearrange("b s v -> (b s) v")

    with tile.TileContext(nc) as tc:
        with (
            tc.tile_pool(name="const", bufs=1) as const,
            tc.tile_pool(name="ktile", bufs=3) as ktile_p,
            tc.tile_pool(name="vtile", bufs=5) as vtile_p,
            tc.tile_pool(name="grp", bufs=2) as grp_p,
            tc.tile_pool(name="qr", bufs=1) as qr_p,
            tc.tile_pool(name="sm", bufs=1) as sm_p,
            tc.tile_pool(name="grow", bufs=3) as grow_p,
            tc.tile_pool(name="misc", bufs=1) as misc,
            tc.tile_pool(name="ps_qb", bufs=2, space="PSUM") as ps_qb,
            tc.tile_pool(name="ps_tr", bufs=2, space="PSUM") as ps_tr,
            tc.tile_pool(name="ps_g", bufs=4, space="PSUM") as ps_g,
        ):
            # ---------------- setup ----------------
            identity = const.tile([128, 128], dt)
            make_identity(nc, identity[:])
            identity_bf = const.tile([128, 128], BF16)
            nc.vector.tensor_copy(out=identity_bf[:], in_=identity[:])
            ones_row = const.tile([1, 128], dt)
            nc.vector.memset(ones_row[:], 1.0)

            iota_i = ktile_p.tile([GRP, S], mybir.dt.int16, tag="ktile")
            nc.gpsimd.iota(iota_i[:], pattern=[[1, S]], base=0, channel_multiplier=0)
            iota_f = const.tile([GRP, S], dt)
            nc.vector.tensor_copy(out=iota_f[:], in_=iota_i[:])

            wqT = const.tile([128, 4, K], dt)
            nc.scalar.dma_start(out=wqT[:], in_=wqT_t[:].rearrange("(c p) k -> p c k", p=128))
            wg1T = const.tile([128, 8, V], dt)
            nc.scalar.dma_start(out=wg1T[:], in_=wg1T_t[:].rearrange("(c p) j -> p c j", p=128))
            wg2T = const.tile([128, 4, V], dt)
            nc.scalar.dma_start(out=wg2T[:], in_=wg2T_t[:].rearrange("(c p) j -> p c j", p=128))
            woT = const.tile([128, 4, V], dt)
            nc.scalar.dma_start(out=woT[:], in_=woT_t[:].rearrange("(c p) j -> p c j", p=128))
            bq_row = const.tile([1, K], dt)
            nc.scalar.dma_start(out=bq_row[:], in_=bq_t[None, :])
            bg1_row = const.tile([1, V], dt)
            nc.scalar.dma_start(out=bg1_row[:], in_=bg1_t[None, :])
            bg2_row = const.tile([1, V], dt)
            nc.scalar.dma_start(out=bg2_row[:], in_=bg2_t[None, :])
            bo_row = const.tile([1, V], dt)
            nc.scalar.dma_start(out=bo_row[:], in_=bo_t[None, :])

            hidden_sb = misc.tile([NB, V], dt)
            nc.scalar.dma_start(out=hidden_sb[:], in_=hidden_t[:, :])
            key_sb = misc.tile([NB, K], dt)
            nc.scalar.dma_start(out=key_sb[:], in_=key_t[:, :])
            value_sb = misc.tile([NB, V], dt)
            nc.scalar.dma_start(out=value_sb[:], in_=value_t[:, :])
            filled_sb = misc.tile([NB, 1], dt)
            nc.scalar.dma_start(out=filled_sb[:], in_=filled_t[:, :])
            wp_sb = misc.tile([NB, 1], dt)
            nc.scalar.dma_start(out=wp_sb[:], in_=wp_t[:, :])
            rowidx_sb = misc.tile([NB, 1], I32)
            nc.scalar.dma_start(out=rowidx_sb[:], in_=rowidx_t[:, :])

            # gather the pre-scatter rows at write_ptr
            kwp_sb = misc.tile([NB, K], FP8)
            vwp_sb = misc.tile([NB, V], FP8)
            if "noind" in _STUBS:
                nc.vector.memset(kwp_sb[:], 0.0)
                nc.vector.memset(vwp_sb[:], 0.0)
            else:
                nc.gpsimd.indirect_dma_start(
                    out=kwp_sb[:], out_offset=None, in_=keys_rows,
                    in_offset=bass.IndirectOffsetOnAxis(ap=rowidx_sb[:, :1], axis=0),
                )
                nc.gpsimd.indirect_dma_start(
                    out=vwp_sb[:], out_offset=None, in_=values_rows,
                    in_offset=bass.IndirectOffsetOnAxis(ap=rowidx_sb[:, :1], axis=0),
                )

            # hiddenT (128v x 64b) chunks
            hT = misc.tile([128, 4, NB], dt)
            for c in range(4):
                tp = ps_tr.tile([128, NB], dt, tag="tr")
                nc.tensor.transpose(out=tp[:], in_=hidden_sb[:, c * 128:(c + 1) * 128], identity=identity[:NB, :NB])
                nc.scalar.copy(out=hT[:, c, :], in_=tp[:])

            # query = hidden @ Wq.T + bq  -> (64b x 128k)
            q_ps = ps_tr.tile([NB, K], dt, tag="tr")
            for c in range(4):
                nc.tensor.matmul(out=q_ps[:], lhsT=hT[:, c, :], rhs=wqT[:, c, :],
                                 start=(c == 0), stop=False)
            nc.tensor.matmul(out=q_ps[:], lhsT=ones_row[:, :NB], rhs=bq_row[:],
                             start=False, stop=True)
            query_sb = misc.tile([NB, K], dt)
            nc.vector.tensor_copy(out=query_sb[:], in_=q_ps[:])

            # raw (unscaled) dot(key_row, query) for old/new rows at write_ptr
            junk_rd = misc.tile([NB, K], dt)
            sold = misc.tile([NB, 1], dt)
            nc.vector.tensor_mul(out=junk_rd[:], in0=kwp_sb[:], in1=query_sb[:])
            nc.vector.tensor_reduce(out=sold[:], in_=junk_rd[:],
                                    axis=mybir.AxisListType.X, op=mybir.AluOpType.add)
            snew = misc.tile([NB, 1], dt)
            nc.vector.tensor_mul(out=junk_rd[:], in0=key_sb[:], in1=query_sb[:])
            nc.vector.tensor_reduce(out=snew[:], in_=junk_rd[:],
                                    axis=mybir.AxisListType.X, op=mybir.AluOpType.add)

            denom0 = misc.tile([NB, 1], dt)
            neg_m_all = misc.tile([NB, 1], dt)
            attnT_groups = []
            g_sb = misc.tile([NB, V], dt)

            prod_s = misc.tile([128, T, K], BF16)

            def scores_stage(g):
                b0 = g * GRP
                # query rows of this group -> partition 0 free-dim layout
                qrows = qr_p.tile([1, GRP * K], dt, tag="qrows")
                if "noqrows" in _STUBS:
                    nc.vector.memset(qrows[:], 0.01)
                else:
                    nc.gpsimd.dma_start(
                        out=qrows[:].rearrange("p (b k) -> p b k", b=GRP),
                        in_=query_sb[b0:b0 + GRP, None, :])
                filled_g = qr_p.tile([GRP, 1], dt, tag="filled_g")
                nc.gpsimd.dma_start(out=filled_g[:], in_=filled_t[b0:b0 + GRP, :])
                penalty_g = sm_p.tile([GRP, S], dt, tag="penalty_g")
                nc.vector.tensor_scalar(
                    out=penalty_g[:], in0=iota_f[:], scalar1=filled_g[:, :1],
                    scalar2=NEG_BIG, op0=mybir.AluOpType.is_ge, op1=mybir.AluOpType.mult)

                # sT layout [128, GRP, T]: reduce writes a packed [T] row per
                # batch (2-byte packed last dim -> DVE 2x eligible)
                sT = grp_p.tile([128, GRP, T], BF16, tag="sT")
                for bl in range(GRP):
                    b = b0 + bl
                    kt = ktile_p.tile([128, T, K], BF16, tag="ktile")
                    nc.gpsimd.dma_start(out=kt[:], in_=keys_view[b])  # fp8->bf16 cast
                    qb = ps_qb.tile([128, 128], dt, tag="qb")
                    nc.tensor.matmul(out=qb[:], lhsT=ones_row[:],
                                     rhs=qrows[:, bl * K:(bl + 1) * K],
                                     start=True, stop=True)
                    qb_sb = ktile_p.tile([128, 128], BF16, tag="qb_sb")
                    nc.scalar.copy(out=qb_sb[:], in_=qb[:])
                    qb_ap = qb_sb[:]
                    qb_bcast = bass.AP(tensor=qb_ap.tensor, offset=qb_ap.offset,
                                       ap=[qb_ap.ap[0], [0, T], qb_ap.ap[1]])
                    nc.vector.tensor_tensor(out=prod_s[:], in0=kt[:], in1=qb_bcast,
                                            op=mybir.AluOpType.mult)
                    with nc.allow_low_precision(reason="bf16 raw scores; /sqrt(K) later"):
                        nc.vector.tensor_reduce(out=sT[:, bl, :], in_=prod_s[:],
                                                axis=mybir.AxisListType.X,
                                                op=mybir.AluOpType.add)

                # transpose score columns back to rows, add the -inf penalty
                scores_g = sm_p.tile([GRP, S], dt, tag="scores_g")
                scores_v = scores_g[:].rearrange("g (x t) -> g x t", t=T)
                penalty_v = penalty_g[:].rearrange("g (x t) -> g x t", t=T)
                for t in range(T):
                    tp = ps_tr.tile([GRP, 128], BF16, tag="tr")
                    nc.tensor.transpose(out=tp[:], in_=sT[:, :, t], identity=identity_bf[:])
                    nc.vector.tensor_tensor(
                        out=scores_v[:, :, t], in0=tp[:],
                        in1=penalty_v[:, :, t],
                        op=mybir.AluOpType.add)

                m_g = sm_p.tile([GRP, 1], dt, tag="m_g")
                nc.vector.tensor_reduce(out=m_g[:], in_=scores_g[:],
                                        axis=mybir.AxisListType.X,
                                        op=mybir.AluOpType.max)
                neg_m_g = sm_p.tile([GRP, 1], dt, tag="neg_m_g")
                nc.scalar.mul(out=neg_m_g[:], in_=m_g[:], mul=-1.0 / SCALE)
                exps_g = sm_p.tile([GRP, S], dt, tag="exps_g")
                denom0_g = sm_p.tile([GRP, 1], dt, tag="denom0_g")
                nc.scalar.activation(
                    out=exps_g[:], in_=scores_g[:],
                    func=mybir.ActivationFunctionType.Exp,
                    bias=neg_m_g[:, :1], scale=1.0 / SCALE,
                    accum_out=denom0_g[:, :1])

                attnT = grp_p.tile([128, T, GRP], VALUES_MM_DTYPE, tag="attnT")
                exps_v = exps_g[:].rearrange("g (x t) -> g x t", t=T)
                for t in range(T):
                    tp = ps_tr.tile([128, GRP], dt, tag="tr")
                    nc.tensor.transpose(out=tp[:],
                                        in_=exps_v[:, :, t],
                                        identity=identity[:GRP, :GRP])
                    nc.scalar.copy(out=attnT[:, t, :], in_=tp[:])
                attnT_groups.append(attnT)

                # stitch per-group scalars into the global (NB,1) tiles
                if "nostitch" not in _STUBS:
                    nc.gpsimd.dma_start(out=denom0[b0:b0 + GRP, :], in_=denom0_g[:])
                    nc.gpsimd.dma_start(out=neg_m_all[b0:b0 + GRP, :], in_=neg_m_g[:])

            def values_stage(g):
                b0 = g * GRP
                attnT = attnT_groups[g]
                for bl in range(GRP):
                    b = b0 + bl
                    vt = vtile_p.tile([128, T, V], VALUES_MM_DTYPE, tag="vtile")
                    nc.sync.dma_start(out=vt[:], in_=values_view[b])
                    g_ps = ps_g.tile([1, V], dt, tag="g_ps")
                    for t in range(T):
                        nc.tensor.matmul(out=g_ps[:], lhsT=attnT[:, t, bl:bl + 1],
                                         rhs=vt[:, t, :],
                                         start=(t == 0), stop=(t == T - 1))
                    g_row = grow_p.tile([1, V], dt, tag="g_row")
                    nc.scalar.copy(out=g_row[:], in_=g_ps[:])
                    if "nogrow" not in _STUBS:
                        nc.gpsimd.dma_start(out=g_sb[b:b + 1, :], in_=g_row[:])

            if "nostitch" in _STUBS:
                nc.vector.memset(denom0[:], 1.0)
                nc.vector.memset(neg_m_all[:], 0.0)
            if "nogrow" in _STUBS or "novals" in _STUBS:
                nc.vector.memset(g_sb[:], 0.0)
            for g in range(NG):
                if g > 0 and "novals" not in _STUBS:
                    values_stage(g - 1)
                scores_stage(g)
            if "novals" not in _STUBS:
                values_stage(NG - 1)

            # ---------------- corrections + softmax denominator ----------------
            eo = misc.tile([NB, 1], dt)
            nc.scalar.activation(out=eo[:], in_=sold[:],
                                 func=mybir.ActivationFunctionType.Exp,
                                 bias=neg_m_all[:, :1], scale=1.0 / SCALE)
            en = misc.tile([NB, 1], dt)
            nc.scalar.activation(out=en[:], in_=snew[:],
                                 func=mybir.ActivationFunctionType.Exp,
                                 bias=neg_m_all[:, :1], scale=1.0 / SCALE)
            mask_wp = misc.tile([NB, 1], dt)
            nc.vector.tensor_tensor(out=mask_wp[:], in0=wp_sb[:], in1=filled_sb[:],
                                    op=mybir.AluOpType.is_lt)
            a_old = misc.tile([NB, 1], dt)
            nc.vector.tensor_mul(out=a_old[:], in0=eo[:], in1=mask_wp[:])
            a_new = misc.tile([NB, 1], dt)
            nc.vector.tensor_mul(out=a_new[:], in0=en[:], in1=mask_wp[:])
            denom = misc.tile([NB, 1], dt)
            nc.vector.tensor_sub(out=denom[:], in0=denom0[:], in1=a_old[:])
            nc.vector.tensor_add(out=denom[:], in0=denom[:], in1=a_new[:])
            recip = misc.tile([NB, 1], dt)
            nc.vector.reciprocal(out=recip[:], in_=denom[:])

            # retrieved = (G + a_new*value - a_old*values[wp]) / denom
            t1 = misc.tile([NB, V], dt)
            nc.vector.tensor_scalar_mul(out=t1[:], in0=value_sb[:], scalar1=a_new[:, :1])
            t2 = misc.tile([NB, V], dt)
            nc.vector.tensor_scalar_mul(out=t2[:], in0=vwp_sb[:], scalar1=a_old[:, :1])
            nc.vector.tensor_sub(out=t1[:], in0=t1[:], in1=t2[:])
            nc.vector.tensor_add(out=t1[:], in0=g_sb[:], in1=t1[:])
            retr = misc.tile([NB, V], dt)
            nc.vector.tensor_scalar_mul(out=retr[:], in0=t1[:], scalar1=recip[:, :1])

            # ---------------- MLP ----------------
            rT = misc.tile([128, 4, NB], dt)
            for c in range(4):
                tp = ps_tr.tile([128, NB], dt, tag="tr")
                nc.tensor.transpose(out=tp[:], in_=retr[:, c * 128:(c + 1) * 128],
                                    identity=identity[:NB, :NB])
                nc.scalar.copy(out=rT[:, c, :], in_=tp[:])

            g_ps = ps_tr.tile([NB, V], dt, tag="tr")
            for ic in range(8):
                lhsT = hT[:, ic, :] if ic < 4 else rT[:, ic - 4, :]
                nc.tensor.matmul(out=g_ps[:], lhsT=lhsT, rhs=wg1T[:, ic, :],
                                 start=(ic == 0), stop=False)
            nc.tensor.matmul(out=g_ps[:], lhsT=ones_row[:, :NB], rhs=bg1_row[:],
                             start=False, stop=True)
            g_act = misc.tile([NB, V], dt)
            nc.scalar.activation(out=g_act[:], in_=g_ps[:],
                                 func=mybir.ActivationFunctionType.Sigmoid)
            nc.vector.tensor_mul(out=g_act[:], in0=g_act[:], in1=g_ps[:])

            gT = misc.tile([128, 4, NB], dt)
            for c in range(4):
                tp = ps_tr.tile([128, NB], dt, tag="tr")
                nc.tensor.transpose(out=tp[:], in_=g_act[:, c * 128:(c + 1) * 128],
                                    identity=identity[:NB, :NB])
                nc.scalar.copy(out=gT[:, c, :], in_=tp[:])

            gate_ps = ps_tr.tile([NB, V], dt, tag="tr")
            for c in range(4):
                nc.tensor.matmul(out=gate_ps[:], lhsT=gT[:, c, :], rhs=wg2T[:, c, :],
                                 start=(c == 0), stop=False)
            nc.tensor.matmul(out=gate_ps[:], lhsT=ones_row[:, :NB], rhs=bg2_row[:],
                             start=False, stop=True)
            gate = misc.tile([NB, V], dt)
            nc.scalar.activation(out=gate[:], in_=gate_ps[:],
                                 func=mybir.ActivationFunctionType.Sigmoid)

            z = misc.tile([NB, V], dt)
            nc.vector.tensor_mul(out=z[:], in0=gate[:], in1=retr[:])
            nc.vector.tensor_add(out=z[:], in0=z[:], in1=hidden_sb[:])

            zT = misc.tile([128, 4, NB], dt)
            for c in range(4):
                tp = ps_tr.tile([128, NB], dt, tag="tr")
                nc.tensor.transpose(out=tp[:], in_=z[:, c * 128:(c + 1) * 128],
                                    identity=identity[:NB, :NB])
                nc.scalar.copy(out=zT[:, c, :], in_=tp[:])

            out_ps = ps_tr.tile([NB, V], dt, tag="tr")
            for c in range(4):
                nc.tensor.matmul(out=out_ps[:], lhsT=zT[:, c, :], rhs=woT[:, c, :],
                                 start=(c == 0), stop=False)
            nc.tensor.matmul(out=out_ps[:], lhsT=ones_row[:, :NB], rhs=bo_row[:],
                             start=False, stop=True)
            out_sb = misc.tile([NB, V], dt)
            nc.vector.tensor_copy(out=out_sb[:], in_=out_ps[:])
            nc.sync.dma_start(out=out_t[:, :], in_=out_sb[:])

    nc.finalize()
    return nc


_NC_CACHE = None


def _get_nc():
    global _NC_CACHE
    if _NC_CACHE is None:
        _NC_CACHE = _build()
    return _NC_CACHE


def _make_in_maps(keys, values, key, value, hidden, write_ptr, filled,
                  Wq, bq, Wg1, bg1, Wg2, bg2, Wo, bo):
    f32 = np.float32
    fp8 = ml_dtypes.float8_e4m3
    keys = np.ascontiguousarray(np.asarray(keys, dtype=f32).astype(fp8))
    values = np.ascontiguousarray(np.asarray(values, dtype=f32).astype(fp8))
    key = np.ascontiguousarray(np.asarray(key, dtype=f32))
    value = np.ascontiguousarray(np.asarray(value, dtype=f32))
    hidden = np.ascontiguousarray(np.asarray(hidden, dtype=f32))
    wp = np.asarray(write_ptr).astype(np.int64)
    fl = np.asarray(filled).astype(np.int64)

    wqT = np.ascontiguousarray(np.asarray(Wq, dtype=f32).T)
    wg1T = np.ascontiguousarray(np.asarray(Wg1, dtype=f32).T)
    wg2T = np.ascontiguousarray(np.asarray(Wg2, dtype=f32).T)
    woT = np.ascontiguousarray(np.asarray(Wo, dtype=f32).T)
    bq = np.ascontiguousarray(np.asarray(bq, dtype=f32))
    bg1 = np.ascontiguousarray(np.asarray(bg1, dtype=f32))
    bg2 = np.ascontiguousarray(np.asarray(bg2, dtype=f32))
    bo = np.ascontiguousarray(np.asarray(bo, dtype=f32))

    filled_w = np.minimum(fl + 1, S).astype(f32).reshape(B, 1)
    wp_f = wp.astype(f32).reshape(B, 1)

    in_maps = []
    for c in range(NCORES):
        sl = slice(c * NB, (c + 1) * NB)
        wp_c = wp[sl]
        row_idx = (np.arange(NB, dtype=np.int64) * S + wp_c).astype(np.int32)
        in_maps.append({
            "keys": keys[sl],
            "values": values[sl],
            "key": key[sl],
            "value": value[sl],
            "hidden": hidden[sl],
            "filled_f": filled_w[sl],
            "wp_f": wp_f[sl],
            "row_idx": row_idx.reshape(NB, 1),
            "WqT": wqT, "Wg1T": wg1T, "Wg2T": wg2T, "WoT": woT,
            "bq": bq, "bg1": bg1, "bg2": bg2, "bo": bo,
        })
    return in_maps


def run(trace=False, **inputs):
    nc = _get_nc()
    in_maps = _make_in_maps(**inputs)
    res = run_bass_kernel_spmd(nc, in_maps, core_ids=list(range(NCORES)),
                               trace=trace)
    out = np.concatenate([res.results[c]["out"] for c in range(NCORES)], axis=0)
    return out, res


def kernel(**inputs) -> np.ndarray:
    out, _ = run(trace=False, **inputs)
    return out



# revision 19
# speedup vs baseline: 2.2558x; 1.3375x over previous
"""EpisodicMemory Trainium2 kernel (8 NeuronCores, pure data parallel over batch).

Reference semantics (per batch b):
    keys_w   = keys   with row write_ptr[b] <- key[b]
    values_w = values with row write_ptr[b] <- value[b]
    filled_w = min(filled + 1, S)
    query    = hidden @ Wq.T + bq
    scores   = (keys_w @ query) / sqrt(K), masked to s < filled_w
    attn     = softmax(scores)
    retrieved= attn @ values_w
    g        = silu([hidden|retrieved] @ Wg1.T + bg1)
    gate     = sigmoid(g @ Wg2.T + bg2)
    out      = (hidden + gate*retrieved) @ Wo.T + bo

The scatter is never materialized: base scores/retrieved are computed from the
original keys/values and corrected algebraically with the gathered old rows at
write_ptr (indirect DMA) plus the new key/value rows.

Perf notes:
  * keys/values stored fp8(e4m3) in DRAM (4x less HBM traffic, rel err ~3e-3).
  * s decomposed into contiguous 128-row chunks; chunks beyond filled_w are
    skipped entirely (DMA + compute). Batches are sorted by filled on the host
    and dealt round-robin to the 8 cores so one SPMD schedule (per-slot chunk
    count = max over cores) is near-optimal and load-balanced.
  * scores batch loop and values matvec of the previous group are interleaved
    at batch granularity so PE/DVE/DMA overlap and the PE stays HAM-warm.
"""

import sys

sys.path.insert(0, "/opt/trn_rl_repo")

import ml_dtypes
import numpy as np

import concourse.bacc as bacc
import concourse.tile as tile
from concourse import bass, mybir
from concourse.bass_utils import run_bass_kernel_spmd
from concourse.masks import make_identity

B, S, K, V = 512, 1024, 128, 512
NCORES = 8
NB = B // NCORES          # 64 batches per core
T = S // 128              # 8 s-chunks of 128
GRP = 16                  # batches per softmax group
NG = NB // GRP            # 4 groups
SCALE = float(np.sqrt(K))
NEG_BIG = -3.0e37

F32 = mybir.dt.float32
I32 = mybir.dt.int32
BF16 = mybir.dt.bfloat16
FP8 = mybir.dt.float8e4


def _build(ncb_slot):
    """ncb_slot: tuple of NB ints — number of live 128-row s-chunks per batch
    slot (same for every core thanks to host-side sorting)."""
    nc = bacc.Bacc()
    dt = F32
    ncg_list = [max(ncb_slot[g * GRP:(g + 1) * GRP]) for g in range(NG)]

    # ---- DRAM tensors (per-core shard) ----
    keys_t = nc.dram_tensor("keys", [NB, S, K], FP8, kind="ExternalInput")
    values_t = nc.dram_tensor("values", [NB, S, V], FP8, kind="ExternalInput")
    key_t = nc.dram_tensor("key", [NB, K], dt, kind="ExternalInput")
    value_t = nc.dram_tensor("value", [NB, V], dt, kind="ExternalInput")
    hidden_t = nc.dram_tensor("hidden", [NB, V], dt, kind="ExternalInput")
    filled_t = nc.dram_tensor("filled_f", [NB, 1], dt, kind="ExternalInput")
    wp_t = nc.dram_tensor("wp_f", [NB, 1], dt, kind="ExternalInput")
    rowidx_t = nc.dram_tensor("row_idx", [NB, 1], I32, kind="ExternalInput")
    wqT_t = nc.dram_tensor("WqT", [V, K], dt, kind="ExternalInput")       # Wq.T
    wg1T_t = nc.dram_tensor("Wg1T", [2 * V, V], dt, kind="ExternalInput")  # Wg1.T
    wg2T_t = nc.dram_tensor("Wg2T", [V, V], dt, kind="ExternalInput")     # Wg2.T
    woT_t = nc.dram_tensor("WoT", [V, V], dt, kind="ExternalInput")       # Wo.T
    bq_t = nc.dram_tensor("bq", [K], dt, kind="ExternalInput")
    bg1_t = nc.dram_tensor("bg1", [V], dt, kind="ExternalInput")
    bg2_t = nc.dram_tensor("bg2", [V], dt, kind="ExternalInput")
    bo_t = nc.dram_tensor("bo", [V], dt, kind="ExternalInput")
    out_t = nc.dram_tensor("out", [NB, V], dt, kind="ExternalOutput")

    keys_rows = keys_t[:].rearrange("b s k -> (b s) k")
    values_rows = values_t[:].rearrange("b s v -> (b s) v")

    with tile.TileContext(nc) as tc:
        with (
            tc.tile_pool(name="const", bufs=1) as const,
            tc.tile_pool(name="ktile", bufs=3) as ktile_p,
            tc.tile_pool(name="vtile", bufs=5) as vtile_p,
            tc.tile_pool(name="grp", bufs=2) as grp_p,
            tc.tile_pool(name="qb4", bufs=2) as qb4_p,
            tc.tile_pool(name="qr", bufs=2) as qr_p,
            tc.tile_pool(name="sm", bufs=1) as sm_p,
            tc.tile_pool(name="grow", bufs=3) as grow_p,
            tc.tile_pool(name="misc", bufs=1) as misc,
            tc.tile_pool(name="ps_qb", bufs=2, space="PSUM") as ps_qb,
            tc.tile_pool(name="ps_tr", bufs=2, space="PSUM") as ps_tr,
            tc.tile_pool(name="ps_g", bufs=4, space="PSUM") as ps_g,
        ):
            # ---------------- setup ----------------
            identity = const.tile([128, 128], dt)
            make_identity(nc, identity[:])
            identity_bf = const.tile([128, 128], BF16)
            nc.vector.tensor_copy(out=identity_bf[:], in_=identity[:])
            ones_row = const.tile([1, 128], dt)
            nc.vector.memset(ones_row[:], 1.0)

            iota_i = ktile_p.tile([GRP, S], mybir.dt.int16, tag="ktile")
            nc.gpsimd.iota(iota_i[:], pattern=[[1, S]], base=0, channel_multiplier=0)
            iota_f = const.tile([GRP, S], dt)
            nc.vector.tensor_copy(out=iota_f[:], in_=iota_i[:])

            wqT = const.tile([128, 4, K], dt)
            nc.scalar.dma_start(out=wqT[:], in_=wqT_t[:].rearrange("(c p) k -> p c k", p=128))
            wg1T = const.tile([128, 8, V], dt)
            nc.scalar.dma_start(out=wg1T[:], in_=wg1T_t[:].rearrange("(c p) j -> p c j", p=128))
            wg2T = const.tile([128, 4, V], dt)
            nc.scalar.dma_start(out=wg2T[:], in_=wg2T_t[:].rearrange("(c p) j -> p c j", p=128))
            woT = const.tile([128, 4, V], dt)
            nc.scalar.dma_start(out=woT[:], in_=woT_t[:].rearrange("(c p) j -> p c j", p=128))
            bq_row = const.tile([1, K], dt)
            nc.scalar.dma_start(out=bq_row[:], in_=bq_t[None, :])
            bg1_row = const.tile([1, V], dt)
            nc.scalar.dma_start(out=bg1_row[:], in_=bg1_t[None, :])
            bg2_row = const.tile([1, V], dt)
            nc.scalar.dma_start(out=bg2_row[:], in_=bg2_t[None, :])
            bo_row = const.tile([1, V], dt)
            nc.scalar.dma_start(out=bo_row[:], in_=bo_t[None, :])

            hidden_sb = misc.tile([NB, V], dt)
            nc.scalar.dma_start(out=hidden_sb[:], in_=hidden_t[:, :])
            key_sb = misc.tile([NB, K], dt)
            nc.scalar.dma_start(out=key_sb[:], in_=key_t[:, :])
            value_sb = misc.tile([NB, V], dt)
            nc.scalar.dma_start(out=value_sb[:], in_=value_t[:, :])
            filled_sb = misc.tile([NB, 1], dt)
            nc.scalar.dma_start(out=filled_sb[:], in_=filled_t[:, :])
            wp_sb = misc.tile([NB, 1], dt)
            nc.scalar.dma_start(out=wp_sb[:], in_=wp_t[:, :])
            rowidx_sb = misc.tile([NB, 1], I32)
            nc.scalar.dma_start(out=rowidx_sb[:], in_=rowidx_t[:, :])

            # gather the pre-scatter rows at write_ptr (fp8, upconverted on read)
            kwp_sb = misc.tile([NB, K], FP8)
            vwp_sb = misc.tile([NB, V], FP8)
            nc.gpsimd.indirect_dma_start(
                out=kwp_sb[:], out_offset=None, in_=keys_rows,
                in_offset=bass.IndirectOffsetOnAxis(ap=rowidx_sb[:, :1], axis=0),
            )
            nc.gpsimd.indirect_dma_start(
                out=vwp_sb[:], out_offset=None, in_=values_rows,
                in_offset=bass.IndirectOffsetOnAxis(ap=rowidx_sb[:, :1], axis=0),
            )

            # hiddenT (128v x 64b) chunks
            hT = misc.tile([128, 4, NB], dt)
            for c in range(4):
                tp = ps_tr.tile([128, NB], dt, tag="tr")
                nc.tensor.transpose(out=tp[:], in_=hidden_sb[:, c * 128:(c + 1) * 128], identity=identity[:NB, :NB])
                nc.scalar.copy(out=hT[:, c, :], in_=tp[:])

            # query = hidden @ Wq.T + bq  -> (64b x 128k)
            q_ps = ps_tr.tile([NB, K], dt, tag="tr")
            for c in range(4):
                nc.tensor.matmul(out=q_ps[:], lhsT=hT[:, c, :], rhs=wqT[:, c, :],
                                 start=(c == 0), stop=False)
            nc.tensor.matmul(out=q_ps[:], lhsT=ones_row[:, :NB], rhs=bq_row[:],
                             start=False, stop=True)
            query_sb = misc.tile([NB, K], dt)
            nc.vector.tensor_copy(out=query_sb[:], in_=q_ps[:])

            # raw (unscaled) dot(key_row, query) for old/new rows at write_ptr
            junk_rd = misc.tile([NB, K], dt)
            sold = misc.tile([NB, 1], dt)
            nc.vector.tensor_mul(out=junk_rd[:], in0=kwp_sb[:], in1=query_sb[:])
            nc.vector.tensor_reduce(out=sold[:], in_=junk_rd[:],
                                    axis=mybir.AxisListType.X, op=mybir.AluOpType.add)
            snew = misc.tile([NB, 1], dt)
            nc.vector.tensor_mul(out=junk_rd[:], in0=key_sb[:], in1=query_sb[:])
            nc.vector.tensor_reduce(out=snew[:], in_=junk_rd[:],
                                    axis=mybir.AxisListType.X, op=mybir.AluOpType.add)

            denom0 = misc.tile([NB, 1], dt)
            neg_m_all = misc.tile([NB, 1], dt)
            attnT_groups = []
            g_sb = misc.tile([NB, V], dt)

            prod_s = misc.tile([128, T, K], BF16)

            def group_head(g):
                b0 = g * GRP
                # this group's query rows into partition-0 free dim
                qrows = qr_p.tile([1, GRP * K], dt, tag="qrows")
                nc.scalar.dma_start(
                    out=qrows[:].rearrange("p (b k) -> p b k", b=GRP),
                    in_=query_sb[b0:b0 + GRP, None, :])
                # broadcast 4 query rows at a time across all 128 partitions
                qb4 = qb4_p.tile([128, 4, 4, K], BF16, tag="qb4")
                for q in range(4):
                    qb_ps = ps_qb.tile([128, 4 * K], dt, tag="qb")
                    nc.tensor.matmul(out=qb_ps[:], lhsT=ones_row[:],
                                     rhs=qrows[:, 4 * q * K:(4 * q + 4) * K],
                                     start=True, stop=True)
                    nc.scalar.copy(out=qb4[:, q, :, :],
                                   in_=qb_ps[:].rearrange("p (j k) -> p j k", j=4))
                filled_g = qb4_p.tile([GRP, 1], dt, tag="filled_g")
                nc.gpsimd.dma_start(out=filled_g[:], in_=filled_t[b0:b0 + GRP, :])
                penalty_g = sm_p.tile([GRP, S], dt, tag="penalty_g")
                nc.vector.tensor_scalar(
                    out=penalty_g[:], in0=iota_f[:], scalar1=filled_g[:, :1],
                    scalar2=NEG_BIG, op0=mybir.AluOpType.is_ge, op1=mybir.AluOpType.mult)
                # sT holds raw scores columns; zero so skipped chunks stay finite
                sT = grp_p.tile([128, GRP, T], BF16, tag="sT")
                nc.vector.memset(sT[:], 0.0)
                return qb4, penalty_g, sT

            def scores_batch(g, bl, qb4, sT):
                b = g * GRP + bl
                ncb = ncb_slot[b]
                kt = ktile_p.tile([128, T, K], BF16, tag="ktile")
                nc.gpsimd.dma_start(
                    out=kt[:, :ncb, :],
                    in_=keys_t[b, :ncb * 128, :].rearrange("(c p) k -> p c k", p=128))
                qb_ap = qb4[:, bl // 4, bl % 4, :]
                qb_bcast = bass.AP(tensor=qb_ap.tensor, offset=qb_ap.offset,
                                   ap=[qb_ap.ap[0], [0, ncb], qb_ap.ap[1]])
                nc.vector.tensor_tensor(out=prod_s[:, :ncb, :], in0=kt[:, :ncb, :],
                                        in1=qb_bcast, op=mybir.AluOpType.mult)
                with nc.allow_low_precision(reason="bf16 raw scores; /sqrt(K) later"):
                    nc.vector.tensor_reduce(out=sT[:, bl, :ncb], in_=prod_s[:, :ncb, :],
                                            axis=mybir.AxisListType.X,
                                            op=mybir.AluOpType.add)

            def group_tail(g, penalty_g, sT):
                b0 = g * GRP
                ncg = ncg_list[g]
                ns = ncg * 128
                # transpose score columns back to rows, add the -inf penalty
                scores_g = sm_p.tile([GRP, S], dt, tag="scores_g")
                for c in range(ncg):
                    tp = ps_tr.tile([GRP, 128], BF16, tag="tr")
                    nc.tensor.transpose(out=tp[:], in_=sT[:, :, c], identity=identity_bf[:])
                    nc.vector.tensor_tensor(
                        out=scores_g[:, c * 128:(c + 1) * 128], in0=tp[:],
                        in1=penalty_g[:, c * 128:(c + 1) * 128],
                        op=mybir.AluOpType.add)

                m_g = sm_p.tile([GRP, 1], dt, tag="m_g")
                nc.vector.tensor_reduce(out=m_g[:], in_=scores_g[:, :ns],
                                        axis=mybir.AxisListType.X,
                                        op=mybir.AluOpType.max)
                neg_m_g = sm_p.tile([GRP, 1], dt, tag="neg_m_g")
                nc.scalar.mul(out=neg_m_g[:], in_=m_g[:], mul=-1.0 / SCALE)
                exps_g = sm_p.tile([GRP, S], dt, tag="exps_g")
                denom0_g = sm_p.tile([GRP, 1], dt, tag="denom0_g")
                nc.scalar.activation(
                    out=exps_g[:, :ns], in_=scores_g[:, :ns],
                    func=mybir.ActivationFunctionType.Exp,
                    bias=neg_m_g[:, :1], scale=1.0 / SCALE,
                    accum_out=denom0_g[:, :1])

                attnT = grp_p.tile([128, T, GRP], FP8, tag="attnT")
                for c in range(ncg):
                    tp = ps_tr.tile([128, GRP], dt, tag="tr")
                    nc.tensor.transpose(out=tp[:],
                                        in_=exps_g[:, c * 128:(c + 1) * 128],
                                        identity=identity[:GRP, :GRP])
                    nc.scalar.copy(out=attnT[:, c, :], in_=tp[:])
                attnT_groups.append(attnT)

                # stitch per-group scalars into the global (NB,1) tiles
                nc.gpsimd.dma_start(out=denom0[b0:b0 + GRP, :], in_=denom0_g[:])
                nc.gpsimd.dma_start(out=neg_m_all[b0:b0 + GRP, :], in_=neg_m_g[:])

            def values_batch(g, bl):
                b = g * GRP + bl
                ncb = ncb_slot[b]
                attnT = attnT_groups[g]
                vt = vtile_p.tile([128, T, V], FP8, tag="vtile")
                nc.sync.dma_start(
                    out=vt[:, :ncb, :],
                    in_=values_t[b, :ncb * 128, :].rearrange("(c p) v -> p c v", p=128))
                g_ps = ps_g.tile([1, V], F32, tag="g_ps")
                for t in range(ncb):
                    nc.tensor.matmul(out=g_ps[:], lhsT=attnT[:, t, bl:bl + 1],
                                     rhs=vt[:, t, :],
                                     start=(t == 0), stop=(t == ncb - 1))
                g_row = grow_p.tile([1, V], dt, tag="g_row")
                nc.scalar.copy(out=g_row[:], in_=g_ps[:])
                nc.scalar.dma_start(out=g_sb[b:b + 1, :], in_=g_row[:])

            for g in range(NG):
                qb4, penalty_g, sT = group_head(g)
                for bl in range(GRP):
                    scores_batch(g, bl, qb4, sT)
                    if g > 0:
                        values_batch(g - 1, bl)
                group_tail(g, penalty_g, sT)
            for bl in range(GRP):
                values_batch(NG - 1, bl)

            # ---------------- corrections + softmax denominator ----------------
            eo = misc.tile([NB, 1], dt)
            nc.scalar.activation(out=eo[:], in_=sold[:],
                                 func=mybir.ActivationFunctionType.Exp,
                                 bias=neg_m_all[:, :1], scale=1.0 / SCALE)
            en = misc.tile([NB, 1], dt)
            nc.scalar.activation(out=en[:], in_=snew[:],
                                 func=mybir.ActivationFunctionType.Exp,
                                 bias=neg_m_all[:, :1], scale=1.0 / SCALE)
            mask_wp = misc.tile([NB, 1], dt)
            nc.vector.tensor_tensor(out=mask_wp[:], in0=wp_sb[:], in1=filled_sb[:],
                                    op=mybir.AluOpType.is_lt)
            a_old = misc.tile([NB, 1], dt)
            nc.vector.tensor_mul(out=a_old[:], in0=eo[:], in1=mask_wp[:])
            a_new = misc.tile([NB, 1], dt)
            nc.vector.tensor_mul(out=a_new[:], in0=en[:], in1=mask_wp[:])
            denom = misc.tile([NB, 1], dt)
            nc.vector.tensor_sub(out=denom[:], in0=denom0[:], in1=a_old[:])
            nc.vector.tensor_add(out=denom[:], in0=denom[:], in1=a_new[:])
            recip = misc.tile([NB, 1], dt)
            nc.vector.reciprocal(out=recip[:], in_=denom[:])

            # retrieved = (G + a_new*value - a_old*values[wp]) / denom
            t1 = misc.tile([NB, V], dt)
            nc.vector.tensor_scalar_mul(out=t1[:], in0=value_sb[:], scalar1=a_new[:, :1])
            t2 = misc.tile([NB, V], dt)
            nc.vector.tensor_scalar_mul(out=t2[:], in0=vwp_sb[:], scalar1=a_old[:, :1])
            nc.vector.tensor_sub(out=t1[:], in0=t1[:], in1=t2[:])
            nc.vector.tensor_add(out=t1[:], in0=g_sb[:], in1=t1[:])
            retr = misc.tile([NB, V], dt)
            nc.vector.tensor_scalar_mul(out=retr[:], in0=t1[:], scalar1=recip[:, :1])

            # ---------------- MLP ----------------
            rT = misc.tile([128, 4, NB], dt)
            for c in range(4):
                tp = ps_tr.tile([128, NB], dt, tag="tr")
                nc.tensor.transpose(out=tp[:], in_=retr[:, c * 128:(c + 1) * 128],
                                    identity=identity[:NB, :NB])
                nc.scalar.copy(out=rT[:, c, :], in_=tp[:])

            g_ps = ps_tr.tile([NB, V], dt, tag="tr")
            for ic in range(8):
                lhsT = hT[:, ic, :] if ic < 4 else rT[:, ic - 4, :]
                nc.tensor.matmul(out=g_ps[:], lhsT=lhsT, rhs=wg1T[:, ic, :],
                                 start=(ic == 0), stop=False)
            nc.tensor.matmul(out=g_ps[:], lhsT=ones_row[:, :NB], rhs=bg1_row[:],
                             start=False, stop=True)
            g_act = misc.tile([NB, V], dt)
            nc.scalar.activation(out=g_act[:], in_=g_ps[:],
                                 func=mybir.ActivationFunctionType.Sigmoid)
            nc.vector.tensor_mul(out=g_act[:], in0=g_act[:], in1=g_ps[:])

            gT = misc.tile([128, 4, NB], dt)
            for c in range(4):
                tp = ps_tr.tile([128, NB], dt, tag="tr")
                nc.tensor.transpose(out=tp[:], in_=g_act[:, c * 128:(c + 1) * 128],
                                    identity=identity[:NB, :NB])
                nc.scalar.copy(out=gT[:, c, :], in_=tp[:])

            gate_ps = ps_tr.tile([NB, V], dt, tag="tr")
            for c in range(4):
                nc.tensor.matmul(out=gate_ps[:], lhsT=gT[:, c, :], rhs=wg2T[:, c, :],
                                 start=(c == 0), stop=False)
            nc.tensor.matmul(out=gate_ps[:], lhsT=ones_row[:, :NB], rhs=bg2_row[:],
                             start=False, stop=True)
            gate = misc.tile([NB, V], dt)
            nc.scalar.activation(out=gate[:], in_=gate_ps[:],
                                 func=mybir.ActivationFunctionType.Sigmoid)

            z = misc.tile([NB, V], dt)
            nc.vector.tensor_mul(out=z[:], in0=gate[:], in1=retr[:])
            nc.vector.tensor_add(out=z[:], in0=z[:], in1=hidden_sb[:])

            zT = misc.tile([128, 4, NB], dt)
            for c in range(4):
                tp = ps_tr.tile([128, NB], dt, tag="tr")
                nc.tensor.transpose(out=tp[:], in_=z[:, c * 128:(c + 1) * 128],
                                    identity=identity[:NB, :NB])
                nc.scalar.copy(out=zT[:, c, :], in_=tp[:])

            out_ps = ps_tr.tile([NB, V], dt, tag="tr")
            for c in range(4):
                nc.tensor.matmul(out=out_ps[:], lhsT=zT[:, c, :], rhs=woT[:, c, :],
                                 start=(c == 0), stop=False)
            nc.tensor.matmul(out=out_ps[:], lhsT=ones_row[:, :NB], rhs=bo_row[:],
                             start=False, stop=True)
            out_sb = misc.tile([NB, V], dt)
            nc.vector.tensor_copy(out=out_sb[:], in_=out_ps[:])
            nc.sync.dma_start(out=out_t[:, :], in_=out_sb[:])

    nc.finalize()
    return nc


_NC_CACHE = {}


def _get_nc(ncb_slot):
    key = tuple(ncb_slot)
    if key not in _NC_CACHE:
        _NC_CACHE[key] = _build(key)
    return _NC_CACHE[key]


def _plan(filled):
    """Sort batches by filled desc, deal round-robin to cores. Returns per-core
    batch index lists and the shared per-slot chunk counts."""
    fl = np.asarray(filled).astype(np.int64)
    filled_w = np.minimum(fl + 1, S)
    order = np.argsort(-filled_w, kind="stable")
    idx = [order[np.arange(NB) * NCORES + c] for c in range(NCORES)]
    ncb = (filled_w + 127) // 128          # ceil, in [1, 8]
    ncb_slot = tuple(int(ncb[order[j * NCORES]]) for j in range(NB))
    return idx, ncb_slot


def _make_in_maps(idx, keys, values, key, value, hidden, write_ptr, filled,
                  Wq, bq, Wg1, bg1, Wg2, bg2, Wo, bo):
    f32 = np.float32
    fp8 = ml_dtypes.float8_e4m3
    keys = np.asarray(keys, dtype=f32).astype(fp8)
    values = np.asarray(values, dtype=f32).astype(fp8)
    key = np.asarray(key, dtype=f32)
    value = np.asarray(value, dtype=f32)
    hidden = np.asarray(hidden, dtype=f32)
    wp = np.asarray(write_ptr).astype(np.int64)
    fl = np.asarray(filled).astype(np.int64)

    wqT = np.ascontiguousarray(np.asarray(Wq, dtype=f32).T)
    wg1T = np.ascontiguousarray(np.asarray(Wg1, dtype=f32).T)
    wg2T = np.ascontiguousarray(np.asarray(Wg2, dtype=f32).T)
    woT = np.ascontiguousarray(np.asarray(Wo, dtype=f32).T)
    bq = np.ascontiguousarray(np.asarray(bq, dtype=f32))
    bg1 = np.ascontiguousarray(np.asarray(bg1, dtype=f32))
    bg2 = np.ascontiguousarray(np.asarray(bg2, dtype=f32))
    bo = np.ascontiguousarray(np.asarray(bo, dtype=f32))

    filled_w = np.minimum(fl + 1, S).astype(f32).reshape(B, 1)
    wp_f = wp.astype(f32).reshape(B, 1)

    in_maps = []
    for c in range(NCORES):
        sl = idx[c]
        wp_c = wp[sl]
        row_idx = (np.arange(NB, dtype=np.int64) * S + wp_c).astype(np.int32)
        in_maps.append({
            "keys": np.ascontiguousarray(keys[sl]),
            "values": np.ascontiguousarray(values[sl]),
            "key": np.ascontiguousarray(key[sl]),
            "value": np.ascontiguousarray(value[sl]),
            "hidden": np.ascontiguousarray(hidden[sl]),
            "filled_f": np.ascontiguousarray(filled_w[sl]),
            "wp_f": np.ascontiguousarray(wp_f[sl]),
            "row_idx": row_idx.reshape(NB, 1),
            "WqT": wqT, "Wg1T": wg1T, "Wg2T": wg2T, "WoT": woT,
            "bq": bq, "bg1": bg1, "bg2": bg2, "bo": bo,
        })
    return in_maps


def run(trace=False, **inputs):
    idx, ncb_slot = _plan(inputs["filled"])
    nc = _get_nc(ncb_slot)
    in_maps = _make_in_maps(idx, **inputs)
    res = run_bass_kernel_spmd(nc, in_maps, core_ids=list(range(NCORES)),
                               trace=trace)
    out = np.empty((B, V), np.float32)
    for c in range(NCORES):
        out[idx[c]] = res.results[c]["out"]
    return out, res


def kernel(**inputs) -> np.ndarray:
    out, _ = run(trace=False, **inputs)
    return out
